# revision 4
# baseline (speedup 1.0000x reference)
"""GAT layer on trn2, v4: two-queue adj streaming (final).

Changes vs v2 (kernel.py):
- adj load split across BOTH hwdge queues (sync + scalar/act): measured
  per-queue-per-partition DMA rate ~0.6-0.8 GB/s stacks across queues,
  giving ~105us for the 16MB adj vs ~155-176us on one queue.
- h + weights load first on sync (scalar starts adj at t=0), so setup
  compute overlaps the adj stream.
- chunks of 4 a-blocks (32KB/part descriptors; 16KB/part chunks collapse
  two-queue throughput). Processing order = arrival order:
  scalar{a0-3}, sync{a8-11}, scalar{a4-7}, sync{a12-15}.
  3 rotating staging buffers (the 4th chunk is queue-serialized behind the
  2nd, so sharing the 1st chunk's buffer costs nothing).
- mx slabs computed per chunk on the fly (saves the 64KB/part mx_all).
- single flat 1MB output store at the end on the scalar queue.

Math identical to v2: row permutation i = 16p + a, i' position = a*128+p;
exp(leaky(s)) factorization rho = e^{0.8 Wh}; G = adj * max(rho_i rho_j, 1)
in fp16; denominator via ones-column on the matmul rhs; LN via
(y-mu)/sqrt(var + eps*D^2) folded into a Prelu activation.
"""
import sys

sys.path.insert(0, "/opt/trn_rl_repo")

import numpy as np

import concourse.bass as bass
import concourse.mybir as mybir
import concourse.tile as tile
from concourse.bass_utils import run_bass_kernel_spmd
from concourse.masks import make_identity

dt = mybir.dt
A = mybir.ActivationFunctionType
Op = mybir.AluOpType

N = 2048
F = 128
T = N // 128      # 16
EPS = 1e-5
ALPHA = 0.2
N_CORES = 8
NB = 4            # a-blocks per chunk


def _fix_sync_waits(nc, max_waits=1):
    """walrus here rejects >1 sync wait per instruction; spill extras onto
    same-engine no-ops inserted just before."""
    k = 0
    for f in nc.m.functions:
        for blk in f.blocks:
            insts = blk.instructions
            i = 0
            while i < len(insts):
                inst = insts[i]
                si = inst.sync_info
                if si is not None and len(si.on_wait) > max_waits:
                    waits = list(si.on_wait)
                    inst.sync_info = mybir.SyncInfo(
                        on_wait=waits[:max_waits], on_update=list(si.on_update))
                    pos = i
                    for w in waits[max_waits:]:
                        nop = mybir.InstNoOp(name=f"wait_spill_{k}", ins=[], outs=[])
                        k += 1
                        nop.engine = inst.engine
                        nop.sync_info = mybir.SyncInfo(on_wait=[w], on_update=[])
                        insts.insert(pos, nop)
                        pos += 1
                        i += 1
                i += 1


def _jstrided(row_ap, jt):
    """Free-dim AP selecting i' positions of natural j-tile jt in ascending
    j order: positions {a*128 + p : p in [8jt, 8jt+8), a in 0..16}."""
    return bass.AP(tensor=row_ap.tensor, offset=row_ap.offset + 8 * jt,
                   ap=[list(row_ap.ap[0]), [1, 8], [128, 16]])


def build_gat_nc(fix_waits=True, reps=1):
    nc = bass.Bass()
    H = nc.dram_tensor("h", [N, F], dt.float32, kind="ExternalInput")
    ADJ = nc.dram_tensor("adj", [N, N], dt.int32, kind="ExternalInput")
    WW = nc.dram_tensor("W_w", [F, F], dt.float32, kind="ExternalInput")
    WB = nc.dram_tensor("W_b", [1, F], dt.float32, kind="ExternalInput")
    AW = nc.dram_tensor("a_w", [2, F], dt.float32, kind="ExternalInput")
    OUT = nc.dram_tensor("out", [N, F], dt.float32, kind="ExternalOutput")

    adj_flat = ADJ[:].rearrange("(p a) j -> p (a j)", p=128) \
                     .rearrange("p (a j) -> p a j", j=N)

    with tile.TileContext(nc) as tc:
      for _rep in range(reps):
        with tc.tile_pool(name="const", bufs=1) as const, \
             tc.tile_pool(name="chunks", bufs=1) as chunks:

            # chunk staging: scalar gets a0-7 as ONE 64KB/part DMA (big
            # descriptors = best two-queue throughput); sync follows h with
            # two 32KB/part chunks.
            def adj_chunk(engine, astart, nblk, tag):
                chi = chunks.tile([128, nblk, N], dt.int32, tag=tag, name=tag)
                engine.dma_start(out=chi[:],
                                 in_=adj_flat[:, astart:astart + nblk, :])
                return chi

            # ---- kick off the adj stream on the scalar queue NOW ----
            chi0 = adj_chunk(nc.scalar, 0, 8, "c0")

            with tc.tile_pool(name="spool", bufs=1) as spool, \
                 tc.tile_pool(name="pp", bufs=2, space="PSUM") as pp:

                ident = const.tile([128, 128], dt.float32)
                make_identity(nc, ident[:])
                identh = const.tile([128, 128], dt.float16)
                make_identity(nc, identh[:])
                ones_row = const.tile([1, 128], dt.float32)
                nc.vector.memset(ones_row[:], 1.0)

                # ---- h + weights on the sync queue ----
                h_flat = spool.tile([128, T, F], dt.float32)
                nc.sync.dma_start(out=h_flat[:],
                                  in_=H[:].rearrange("(p a) f -> p (a f)", p=128))
                Ww_sb = const.tile([F, F], dt.float32)
                nc.sync.dma_start(out=Ww_sb[:], in_=WW[:])
                Wb_row = const.tile([1, F], dt.float32)
                nc.sync.dma_start(out=Wb_row[:], in_=WB[:])
                aw_sb = const.tile([2, F], dt.float32)
                nc.sync.dma_start(out=aw_sb[:], in_=AW[:])

                # sync queue joins the adj stream behind the weights
                chi1 = adj_chunk(nc.sync, 8, 8, "c1")

                # ---- W_b column via PE transpose of the [1,128] row ----
                pwb = pp.tile([128, 1], dt.float32, tag="ps", name="ps")
                nc.tensor.transpose(out=pwb[:], in_=Wb_row[0:1, :],
                                    identity=ident[0:1, 0:1])
                Wb_col = const.tile([F, 1], dt.float32)
                nc.scalar.activation(out=Wb_col[:], in_=pwb[:], func=A.Identity)

                # ---- hT[f, i'] via PE transpose of each a-block ----
                hT = spool.tile([128, N], dt.float32)
                for a in range(T):
                    pt = pp.tile([128, 128], dt.float32, tag="ps", name="ps")
                    nc.tensor.transpose(out=pt[:], in_=h_flat[:, a, :],
                                        identity=ident[:])
                    nc.scalar.activation(out=hT[:, a * 128:(a + 1) * 128], in_=pt[:],
                                         func=A.Identity)
                ptw = pp.tile([128, 128], dt.float32, tag="ps", name="ps")
                nc.tensor.transpose(out=ptw[:], in_=Ww_sb[:], identity=ident[:])
                WwT = const.tile([128, 128], dt.float32)
                nc.scalar.activation(out=WwT[:], in_=ptw[:], func=A.Identity)

                # ---- WhT[o, i'] = Ww @ h^T + b ----
                WhT = spool.tile([128, N], dt.float32)
                for c in range(4):
                    pw = pp.tile([128, 512], dt.float32, tag="ps", name="ps")
                    nc.tensor.matmul(out=pw[:], lhsT=WwT[:],
                                     rhs=hT[:, c * 512:(c + 1) * 512],
                                     start=True, stop=True)
                    nc.scalar.activation(out=WhT[:, c * 512:(c + 1) * 512], in_=pw[:],
                                         func=A.Identity, bias=Wb_col[:], scale=1.0)

                # ---- Wh1/Wh2 rows (i'-ordered) ----
                pa = pp.tile([128, 2], dt.float32, tag="ps", name="ps")
                nc.tensor.transpose(out=pa[:], in_=aw_sb[:], identity=ident[0:2, 0:2])
                acols = const.tile([128, 2], dt.float32)
                nc.scalar.activation(out=acols[:], in_=pa[:], func=A.Identity)

                rows = [spool.tile([1, N], dt.float32, tag=f"row{r}", name=f"row{r}")
                        for r in range(2)]
                for r in range(2):
                    for c in range(4):
                        pr = pp.tile([1, 512], dt.float32, tag="ps", name="ps")
                        nc.tensor.matmul(out=pr[:], lhsT=acols[:, r:r + 1],
                                         rhs=WhT[:, c * 512:(c + 1) * 512],
                                         start=True, stop=True)
                        nc.scalar.activation(out=rows[r][0:1, c * 512:(c + 1) * 512],
                                             in_=pr[:], func=A.Identity)

                # ---- Wh2 columns in natural-j order (strided-AP transposes) ----
                colsp = pp.tile([128, 16], dt.float32, tag="ps", name="ps")
                w2tmp = spool.tile([1, N], dt.float32)
                for jt in range(T):
                    nc.scalar.activation(out=w2tmp[0:1, jt * 128:(jt + 1) * 128],
                                         in_=_jstrided(rows[1][0:1, :], jt),
                                         func=A.Identity)
                for jt in range(T):
                    nc.tensor.transpose(out=colsp[:, jt:jt + 1],
                                        in_=w2tmp[0:1, jt * 128:(jt + 1) * 128],
                                        identity=ident[0:1, 0:1])
                wh2cols = const.tile([128, 16], dt.float32)
                nc.scalar.activation(out=wh2cols[:], in_=colsp[:], func=A.Identity)

                # ---- per-j factors ----
                rj_sb = const.tile([128, 16], dt.float32)
                nc.scalar.activation(out=rj_sb[:], in_=wh2cols[:], func=A.Exp,
                                     scale=0.8)
                rjinv = const.tile([128, 16], dt.float32)
                nc.scalar.activation(out=rjinv[:], in_=wh2cols[:], func=A.Exp,
                                     scale=-0.8)
                ew2cols = const.tile([128, 16], dt.float32)
                nc.scalar.activation(out=ew2cols[:], in_=wh2cols[:], func=A.Exp,
                                     scale=0.2)

                # ---- rho_i broadcast [128(bcast), i'] fp16 ----
                rib = const.tile([128, N], dt.float16)
                for c in range(4):
                    pb = pp.tile([128, 512], dt.float32, tag="ps", name="ps")
                    nc.tensor.matmul(out=pb[:], lhsT=ones_row[0:1, :],
                                     rhs=rows[0][0:1, c * 512:(c + 1) * 512],
                                     start=True, stop=True)
                    nc.scalar.activation(out=rib[:, c * 512:(c + 1) * 512], in_=pb[:],
                                         func=A.Exp, scale=0.8)

                # ---- R tiles [j-part, 0:128]=e^{.2Wh2}*Wh, col 128=e^{.2Wh2} ----
                R_sb = const.tile([128, T, 132], dt.float16)
                for jt in range(T):
                    whjt = spool.tile([128, 128], dt.float32, tag="whjt",
                                      name="whjt", bufs=2)
                    nc.scalar.activation(out=whjt[:], in_=_jstrided(WhT[:], jt),
                                         func=A.Identity)
                    pR = pp.tile([128, 128], dt.float32, tag="ps", name="ps")
                    nc.tensor.transpose(out=pR[:], in_=whjt[:],
                                        identity=ident[:])
                    nc.scalar.activation(out=R_sb[:, jt, 0:128], in_=pR[:],
                                         func=A.Identity,
                                         scale=ew2cols[:, jt:jt + 1])
                    nc.vector.tensor_copy(out=R_sb[:, jt, 128:129],
                                          in_=ew2cols[:, jt:jt + 1])

            # ---- main loop over chunks in arrival order ----
            with tc.tile_pool(name="chunksh", bufs=3) as chunksh, \
                 tc.tile_pool(name="mxc", bufs=2) as mxc, \
                 tc.tile_pool(name="gp", bufs=2) as gp, \
                 tc.tile_pool(name="stagp", bufs=2, space="PSUM") as stagp, \
                 tc.tile_pool(name="accp", bufs=3, space="PSUM") as accp, \
                 tc.tile_pool(name="lnp", bufs=8) as lnp, \
                 tc.tile_pool(name="outf", bufs=1) as outf:

                out_flat = outf.tile([128, T, F], dt.float32)

                groups = [(chi0, 0, s) for s in range(4)] + \
                         [(chi1, 8, s) for s in range(4)]
                for chi, cstart, s in groups:
                    gstart = cstart + 2 * s   # first a-block of this group

                    # mx slab for this group's i'-range:
                    # mx[j-part, jt, i'] = max(rho_i rho_j, 1) / rho_j
                    mxch = mxc.tile([128, T, 256], dt.float16,
                                    tag="mx", name="mx")
                    lo = gstart * 128
                    for jt in range(T):
                        nc.vector.tensor_scalar(out=mxch[:, jt, :],
                                                in0=rib[:, lo:lo + 256],
                                                scalar1=rjinv[:, jt:jt + 1],
                                                scalar2=rj_sb[:, jt:jt + 1],
                                                op0=Op.max, op1=Op.mult)

                    # cast this 2-block group
                    ch = chunksh.tile([128, 2, N], dt.float16,
                                      tag="ch", name="ch")
                    nc.gpsimd.tensor_copy(out=ch[:],
                                          in_=chi[:, 2 * s:2 * s + 2, :])

                    for ail in range(2):
                        a = gstart + ail
                        acc = accp.tile([128, 132], dt.float32, tag="acc",
                                        name="acc")
                        g = gp.tile([128, T, 128], dt.float16, tag="g", name="g")
                        for half in range(2):
                            stag = stagp.tile([128, 1024], dt.float16, tag="stag",
                                              name="stag")
                            for k in range(8):
                                jt = half * 8 + k
                                nc.tensor.transpose(
                                    out=stag[:, k * 128:(k + 1) * 128],
                                    in_=ch[:, ail, jt * 128:(jt + 1) * 128],
                                    identity=identh[:])
                            stag3 = stag[:].rearrange("p (k q) -> p k q", q=128)
                            nc.vector.tensor_tensor(
                                out=g[:, half * 8:(half + 1) * 8, :], in0=stag3,
                                in1=mxch[:, half * 8:(half + 1) * 8,
                                         ail * 128:(ail + 1) * 128],
                                op=Op.mult)
                        for jt in range(T):
                            nc.tensor.matmul(out=acc[:, 0:129], lhsT=g[:, jt, :],
                                             rhs=R_sb[:, jt, 0:129],
                                             start=(jt == 0), stop=(jt == T - 1))

                        # ---- layernorm + leaky into out_flat[:, a, :] ----
                        y = acc[:, 0:128]
                        Dc = acc[:, 128:129]
                        stats = lnp.tile([128, 6], dt.float32, tag="stats",
                                         name="st")
                        nc.vector.bn_stats(out=stats[:], in_=y)
                        mv = lnp.tile([128, 2], dt.float32, tag="mv", name="mv")
                        nc.vector.bn_aggr(out=mv[:], in_=stats[:])
                        dsb = lnp.tile([128, 1], dt.float32, tag="dsb", name="dsb")
                        nc.vector.tensor_copy(out=dsb[:], in_=Dc)
                        d2e = lnp.tile([128, 1], dt.float32, tag="d2e", name="d2e")
                        nc.vector.scalar_tensor_tensor(out=d2e[:], in0=dsb[:],
                                                       scalar=EPS, in1=dsb[:],
                                                       op0=Op.mult, op1=Op.mult)
                        lnv = lnp.tile([128, 1], dt.float32, tag="lnv", name="lnv")
                        nc.scalar.activation(out=lnv[:], in_=mv[:, 1:2], func=A.Ln,
                                             bias=d2e[:], scale=1.0)
                        rs = lnp.tile([128, 1], dt.float32, tag="rs", name="rs")
                        nc.scalar.activation(out=rs[:], in_=lnv[:], func=A.Exp,
                                             scale=-0.5)
                        nmrs = lnp.tile([128, 1], dt.float32, tag="nmrs", name="nm")
                        nc.vector.tensor_scalar(out=nmrs[:], in0=mv[:, 0:1],
                                                scalar1=rs[:, 0:1], scalar2=-1.0,
                                                op0=Op.mult, op1=Op.mult)
                        nc.scalar.activation(out=out_flat[:, a, :], in_=y,
                                             func=A.Prelu, bias=nmrs[:],
                                             scale=rs[:, 0:1], alpha=ALPHA)

                # ---- one flat output store (scalar queue: finished first) ----
                nc.scalar.dma_start(
                    out=OUT[:].rearrange("(p a) f -> p (a f)", p=128),
                    in_=out_flat[:].rearrange("p a f -> p (a f)"))

    if fix_waits:
        _fix_sync_waits(nc)
    return nc


_NC_CACHE = None


def _get_nc():
    global _NC_CACHE
    if _NC_CACHE is None:
        _NC_CACHE = build_gat_nc()
    return _NC_CACHE


def kernel(h, adj, W_w, W_b, a_w):
    h = np.ascontiguousarray(np.asarray(h, dtype=np.float32))
    adj = np.ascontiguousarray(np.asarray(adj, dtype=np.int32))
    W_w = np.ascontiguousarray(np.asarray(W_w, dtype=np.float32))
    W_b = np.ascontiguousarray(np.asarray(W_b, dtype=np.float32)).reshape(1, F)
    a_w = np.ascontiguousarray(np.asarray(a_w, dtype=np.float32)).reshape(2, F)

    B = h.shape[0]
    assert B == N_CORES and h.shape == (B, N, F) and adj.shape == (B, N, N)

    nc = _get_nc()
    in_maps = [
        {"h": h[b], "adj": adj[b], "W_w": W_w, "W_b": W_b, "a_w": a_w}
        for b in range(B)
    ]
    res = run_bass_kernel_spmd(nc, in_maps, core_ids=list(range(N_CORES)))
    return np.stack([res.results[b]["out"] for b in range(B)], axis=0)
